# revision 35
# baseline (speedup 1.0000x reference)
"""Trainium2 Bass kernel for nn_Mlp_moe: dense patch-token MLP + top-1 gated
atom (expert) routing for 6 CLS task tokens.

Sharding over 8 NeuronCores:
  - Patch MLP: data-parallel over batch B=64 -> 8 batches (1568 patch tokens)
    per core, weights replicated in SBUF.
  - Atom/CLS part: hidden dim H=3072 sharded 8-way; every core processes all
    384 CLS tokens on its H-shard and emits a partial output summed on the
    host. Routing is computed on the host and realized by permuting the CLS
    tokens so that each src atom's tokens are contiguous (grouped GEMMs, no
    all-atom masking).

Patch GEMMs run as fp8 (e4m3) DoubleRow matmuls with a 3-term hi/lo
decomposition: operands split as v = vh + vl (two fp8 values ~ 9 mantissa
bits); out = xh.wh + xl.wh + xh.wl recovers bf16-level accuracy while the
DoubleRow pairs contract 2 values/cell/cycle (~1.8x bf16 on the wh pass).
Weights are pre-scaled by 64 to clear the fp8 subnormal range; the inverse
scale folds into the gelu (activation scale) and the output staging multiply.

Schedule: PE warm-up dummies from program start; DMAs in global need order
with at most 3 early issues on ScalarE (an engine's next dma_start blocks
until its previous transfer completes, and ScalarE must be free to run
gelus); w1 arrives in 8 pieces striped across queues; chunk pairs {0,1} and
{2,3} are processed together so each stationary weight pair serves 6
matmuls.
"""

import numpy as np
import ml_dtypes

import concourse.bass as bass
import concourse.bacc as bacc
import concourse.mybir as mybir
from concourse import tile
from concourse.bass_utils import run_bass_kernel_spmd

NCORES = 8
B, NCLS, P, D, H = 64, 6, 196, 768, 3072
NA = 5
HSH = H // NCORES            # 384: per-core atom hidden shard
BPC = B // NCORES            # 8 batches per core
TPC = BPC * P                # 1568 patch tokens per core
NT = B * NCLS                # 384 cls tokens
DT = D // 128                # 6 d-tiles
HT = H // 128                # 24 h-tiles
NG = DT // 2                 # 3 DoubleRow d-pair groups (K=256 each)
NJ = HT // 2                 # 12 DoubleRow h-pair groups
HLT = NA * HSH // 128        # 15 atom h-shard tiles (a-major, 3 per atom)
KPA = HSH // 128             # 3 h-shard tiles per atom
CW = 392
NCH = 4
NW1P = 8                     # w1 DMA pieces (3 h-tiles each)
HPP = HT // NW1P             # 3 h-tiles per piece
NWARM = 30                   # PE warm-up dummy matmuls
WSC = 64.0                   # fp8 weight pre-scale (clears subnormals)

LEFT_KEYS = np.array([3, 4, 8, 9, 13, 14], dtype=np.int64)
RIGHT_KEYS = np.array([15, 20, 16, 21, 17, 22], dtype=np.int64)

BF16 = mybir.dt.bfloat16
F32 = mybir.dt.float32
F8 = mybir.dt.float8e4
PM = mybir.MatmulPerfMode.DoubleRow
AF = mybir.ActivationFunctionType

_CACHE = {}
LAST_RESULTS = None  # BassKernelResults of the most recent run (for profiling)


def _build_program(goff, dranges):
    """goff: 6 cumulative offsets of the 5 src-atom token groups. dranges[a]:
    (start, end) column ranges whose tokens route their output via atom a.
    """
    nc = bacc.Bacc(None, target_bir_lowering=False, debug=False,
                   num_devices=NCORES)

    xP_d = nc.dram_tensor("xP", [128, NCH, 2, NG, 2, CW], F8,
                          kind="ExternalInput")
    w1P_d = nc.dram_tensor("w1P", [128, NW1P, 2, HPP, NG, 2, 128], F8,
                           kind="ExternalInput")
    b1T_d = nc.dram_tensor("b1T", [128, HT], F32, kind="ExternalInput")
    w2P_d = nc.dram_tensor("w2P", [128, 2, 2, 3, NJ, 2, 128], F8,
                           kind="ExternalInput")
    clsT_d = nc.dram_tensor("clsT", [128, DT * NT], BF16,
                            kind="ExternalInput")
    ainT_d = nc.dram_tensor("ainT", [DT, 128, NA * HSH], BF16,
                            kind="ExternalInput")
    ainbT_d = nc.dram_tensor("ainbT", [128, HLT], F32, kind="ExternalInput")
    aoutT_d = nc.dram_tensor("aoutT", [NA, 128, KPA * D], BF16,
                             kind="ExternalInput")
    wrep_d = nc.dram_tensor("wrep", [128, NT], BF16, kind="ExternalInput")
    poutT_d = nc.dram_tensor("poutT", [DT, 128, TPC], BF16,
                             kind="ExternalOutput")
    cpartT_d = nc.dram_tensor("cpartT", [DT, 128, NT], BF16,
                              kind="ExternalOutput")

    with tile.TileContext(nc) as tc:
        with (
            tc.tile_pool(name="w", bufs=1) as wp,
            tc.tile_pool(name="gat", bufs=1) as gp,
            tc.tile_pool(name="hida", bufs=1) as hp,
            tc.tile_pool(name="xin", bufs=2) as xp,
            tc.tile_pool(name="g1", bufs=48) as g1p,
            tc.tile_pool(name="tmp", bufs=4) as tp,
            tc.tile_pool(name="ostg", bufs=4) as op,
            tc.tile_pool(name="ps", bufs=7, space="PSUM") as pp,
            tc.tile_pool(name="psw", bufs=1, space="PSUM") as pwp,
        ):
            # ---- PE warm-up: dummy matmuls from program start ----
            wdum = wp.tile([128, 512], BF16, tag="wdum", name="wdum")
            nc.vector.memset(wdum[:], 0.03125)
            pdum = pwp.tile([128, 256], F32, tag="pdum", name="pdum")
            for _ in range(NWARM):
                nc.tensor.matmul(pdum[:], wdum[:, :128], wdum[:, :256],
                                 start=True, stop=True)

            # ---- tiles ----
            w1t = [wp.tile([128, 2, HPP, NG, 2, 128], F8, tag=f"w1{q}",
                           name=f"w1{q}") for q in range(NW1P)]
            b1T = wp.tile([128, HT], F32, tag="b1", name="b1")
            w2t = wp.tile([128, 2, DT, NJ, 2, 128], F8, tag="w2", name="w2")
            clsT = wp.tile([128, DT * NT], BF16, tag="cls", name="cls")
            ainbT = wp.tile([128, HLT], F32, tag="ainb", name="ainb")
            ainT = [wp.tile([128, NA * HSH], BF16, tag=f"ain{d}",
                            name=f"ain{d}") for d in range(DT)]
            wrep = wp.tile([128, NT], BF16, tag="wr", name="wr")
            aoutT = [wp.tile([128, KPA * D], BF16, tag=f"ao{a}",
                             name=f"ao{a}") for a in range(NA)]

            def xtile():
                return xp.tile([128, 2, NG, 2, CW], F8, tag="x", name="x")

            # ---- DMA issues, global need order ----
            # ScalarE gets only 3 early issues (it must be free for gelus);
            # sync/gpsimd carry the rest in need order.
            xt0, xt1 = xtile(), xtile()
            nc.scalar.dma_start(xt0[:, :, 0], xP_d[:, 0, :, 0])
            nc.sync.dma_start(xt0[:, :, 1], xP_d[:, 0, :, 1])
            nc.gpsimd.dma_start(xt0[:, :, 2], xP_d[:, 0, :, 2])
            nc.sync.dma_start(xt1[:, :, 0], xP_d[:, 1, :, 0])
            nc.gpsimd.dma_start(xt1[:, :, 1], xP_d[:, 1, :, 1])
            nc.scalar.dma_start(b1T[:], b1T_d[:])
            nc.sync.dma_start(xt1[:, :, 2], xP_d[:, 1, :, 2])
            # w1 piece 0 in hh-thirds (one per queue), rest in s-halves
            nc.scalar.dma_start(w1t[0][:, :, 0], w1P_d[:, 0, :, 0])
            nc.gpsimd.dma_start(w1t[0][:, :, 1], w1P_d[:, 0, :, 1])
            nc.sync.dma_start(w1t[0][:, :, 2], w1P_d[:, 0, :, 2])
            for p in range(1, NW1P):
                h1, h2 = (nc.sync, nc.gpsimd) if p % 2 else \
                    (nc.gpsimd, nc.sync)
                h1.dma_start(w1t[p][:, 0], w1P_d[:, p, 0])
                h2.dma_start(w1t[p][:, 1], w1P_d[:, p, 1])
            # wave 2: atom-in tensors
            nc.gpsimd.dma_start(ainT[0][:], ainT_d[0])
            nc.sync.dma_start(clsT[:], clsT_d[:])
            nc.gpsimd.dma_start(ainT[1][:], ainT_d[1])
            nc.sync.dma_start(ainT[2][:], ainT_d[2])
            nc.gpsimd.dma_start(ainT[3][:], ainT_d[3])
            nc.sync.dma_start(ainT[4][:], ainT_d[4])
            nc.gpsimd.dma_start(ainT[5][:], ainT_d[5])
            nc.sync.dma_start(ainbT[:], ainbT_d[:])
            # wave 3: w2 + gate weights + atom-out tensors
            nc.gpsimd.dma_start(w2t[:, :, 0:3], w2P_d[:, 0])
            nc.sync.dma_start(w2t[:, :, 3:6], w2P_d[:, 1])
            nc.gpsimd.dma_start(wrep[:], wrep_d[:])
            nc.sync.dma_start(aoutT[0][:], aoutT_d[0])
            nc.gpsimd.dma_start(aoutT[1][:], aoutT_d[1])
            nc.sync.dma_start(aoutT[2][:], aoutT_d[2])
            nc.gpsimd.dma_start(aoutT[3][:], aoutT_d[3])
            nc.sync.dma_start(aoutT[4][:], aoutT_d[4])

            # ---- patch GEMM1 for a chunk pair (3-term fp8 DoubleRow) ----
            # psum = 64*h1; gelu applies the 1/64 via activation scale.
            def patch_g1_fp8(xtA, xtB):
                gh = {}
                gl = {}
                for h in range(HT):
                    q, hh = divmod(h, HPP)
                    j, i = divmod(h, 2)
                    psA = pp.tile([128, 512], F32, tag="ps", name="ps")
                    psB = pp.tile([128, 512], F32, tag="ps", name="ps")
                    for g in range(NG):
                        lh = w1t[q][:, 0, hh, g]
                        ll = w1t[q][:, 1, hh, g]
                        nc.tensor.matmul(psA[:, :CW], lh, xtA[:, 0, g],
                                         start=(g == 0), stop=False,
                                         perf_mode=PM)
                        nc.tensor.matmul(psA[:, :CW], lh, xtA[:, 1, g],
                                         start=False, stop=False,
                                         perf_mode=PM)
                        nc.tensor.matmul(psB[:, :CW], lh, xtB[:, 0, g],
                                         start=(g == 0), stop=False,
                                         perf_mode=PM)
                        nc.tensor.matmul(psB[:, :CW], lh, xtB[:, 1, g],
                                         start=False, stop=False,
                                         perf_mode=PM)
                        nc.tensor.matmul(psA[:, :CW], ll, xtA[:, 0, g],
                                         start=False, stop=(g == NG - 1),
                                         perf_mode=PM)
                        nc.tensor.matmul(psB[:, :CW], ll, xtB[:, 0, g],
                                         start=False, stop=(g == NG - 1),
                                         perf_mode=PM)
                    for cc, ps in ((0, psA), (1, psB)):
                        if i == 0:
                            gh[(cc, j)] = g1p.tile([128, 2, CW], F8,
                                                   tag="g1", name="g1")
                            gl[(cc, j)] = g1p.tile([128, 2, CW], F8,
                                                   tag="g1", name="g1")
                        tmp = tp.tile([128, CW], BF16, tag="tmp", name="tmp")
                        nc.scalar.activation(tmp[:], ps[:, :CW], AF.Gelu,
                                             bias=b1T[:, h:h + 1],
                                             scale=1.0 / WSC)
                        nc.vector.tensor_copy(gh[(cc, j)][:, i], tmp[:])
                        nc.vector.tensor_sub(gl[(cc, j)][:, i], tmp[:],
                                             gh[(cc, j)][:, i])
                return gh, gl

            def patch_g2_fp8(cA, cB, gh, gl):
                for dp in range(DT):
                    psA = pp.tile([128, 512], F32, tag="ps", name="ps")
                    psB = pp.tile([128, 512], F32, tag="ps", name="ps")
                    for j in range(NJ):
                        lh = w2t[:, 0, dp, j]
                        ll = w2t[:, 1, dp, j]
                        nc.tensor.matmul(psA[:, :CW], lh, gh[(0, j)][:],
                                         start=(j == 0), stop=False,
                                         perf_mode=PM)
                        nc.tensor.matmul(psA[:, :CW], lh, gl[(0, j)][:],
                                         start=False, stop=False,
                                         perf_mode=PM)
                        nc.tensor.matmul(psB[:, :CW], lh, gh[(1, j)][:],
                                         start=(j == 0), stop=False,
                                         perf_mode=PM)
                        nc.tensor.matmul(psB[:, :CW], lh, gl[(1, j)][:],
                                         start=False, stop=False,
                                         perf_mode=PM)
                        nc.tensor.matmul(psA[:, :CW], ll, gh[(0, j)][:],
                                         start=False, stop=(j == NJ - 1),
                                         perf_mode=PM)
                        nc.tensor.matmul(psB[:, :CW], ll, gh[(1, j)][:],
                                         start=False, stop=(j == NJ - 1),
                                         perf_mode=PM)
                    for cc, ps in ((cA, psA), (cB, psB)):
                        stg = op.tile([128, CW], BF16, tag="ostg",
                                      name="ostg")
                        nc.vector.tensor_scalar_mul(stg[:], ps[:, :CW],
                                                    1.0 / WSC)
                        nc.gpsimd.dma_start(
                            poutT_d[dp][:, cc * CW:(cc + 1) * CW], stg[:])

            gh01, gl01 = patch_g1_fp8(xt0, xt1)

            # ---- phase A: grouped atom in-GEMM + gelu (bf16) ----
            Gk = [gp.tile([128, NT], BF16, tag=f"g{k}", name=f"g{k}")
                  for k in range(KPA)]
            for s in range(NA):
                o0, o1 = goff[s], goff[s + 1]
                ns = o1 - o0
                if ns == 0:
                    continue
                for k in range(KPA):
                    ps = pp.tile([128, 512], F32, tag="ps", name="ps")
                    c0 = s * HSH + k * 128
                    for d in range(DT):
                        nc.tensor.matmul(
                            ps[:, :ns],
                            ainT[d][:, c0:c0 + 128],
                            clsT[:, d * NT + o0:d * NT + o1],
                            start=(d == 0), stop=(d == DT - 1))
                    nc.scalar.activation(Gk[k][:, o0:o1], ps[:, :ns],
                                         AF.Gelu,
                                         bias=ainbT[:, s * KPA + k:
                                                    s * KPA + k + 1])

            # ---- phase B: scale hidden by the gate weight (DVE) ----
            Hk = []
            for k in range(KPA):
                h = hp.tile([128, NT], BF16, tag=f"hid{k}", name=f"hid{k}")
                nc.vector.tensor_mul(h[:], Gk[k][:], wrep[:])
                Hk.append(h)

            patch_g2_fp8(0, 1, gh01, gl01)

            # ---- atom out-GEMM, grouped by dst atom ----
            # PSUM zero regions are whole banks: only the first matmul into
            # the tile carries start=True; later writes rely on the bank's
            # pending-zero for their first touch.
            nmm_out = sum(KPA * len(dranges[a]) for a in range(NA))
            for dp in range(DT):
                ps = pp.tile([128, 512], F32, tag="ps", name="ps")
                n = 0
                for a in range(NA):
                    for k in range(KPA):
                        for (r0, r1) in dranges[a]:
                            nc.tensor.matmul(
                                ps[:, r0:r1],
                                aoutT[a][:, k * D + dp * 128:
                                         k * D + (dp + 1) * 128],
                                Hk[k][:, r0:r1],
                                start=(n == 0), stop=(n == nmm_out - 1),
                                skip_group_check=True)
                            n += 1
                stg = op.tile([128, CW], BF16, tag="cstg", name="cstg")
                nc.vector.tensor_copy(stg[:, :NT], ps[:, :NT])
                nc.gpsimd.dma_start(cpartT_d[dp], stg[:, :NT])

            # ---- patch chunk pair {2,3} ----
            xt2, xt3 = xtile(), xtile()
            nc.sync.dma_start(xt2[:], xP_d[:, 2])
            nc.gpsimd.dma_start(xt3[:], xP_d[:, 3])
            gh23, gl23 = patch_g1_fp8(xt2, xt3)
            patch_g2_fp8(2, 3, gh23, gl23)

    nc.compile()
    return nc


def _sigmoid(x):
    out = np.empty_like(x)
    pos = x >= 0
    out[pos] = 1.0 / (1.0 + np.exp(-x[pos]))
    ex = np.exp(x[~pos])
    out[~pos] = ex / (1.0 + ex)
    return out


def _split8(a):
    e4 = ml_dtypes.float8_e4m3
    hi = a.astype(e4)
    lo = (a - hi.astype(np.float32)).astype(e4)
    return hi, lo


def kernel(x, patch_w1, patch_b1, patch_w2, patch_b2, gate_delta,
           atom_in_w, atom_in_b, atom_out_w, atom_out_b):
    x = np.asarray(x, dtype=np.float32)
    patch_w1 = np.asarray(patch_w1, dtype=np.float32)
    patch_b1 = np.asarray(patch_b1, dtype=np.float32)
    patch_w2 = np.asarray(patch_w2, dtype=np.float32)
    patch_b2 = np.asarray(patch_b2, dtype=np.float32)
    gate_delta = np.asarray(gate_delta, dtype=np.float32)
    atom_in_w = np.asarray(atom_in_w, dtype=np.float32)
    atom_in_b = np.asarray(atom_in_b, dtype=np.float32)
    atom_out_w = np.asarray(atom_out_w, dtype=np.float32)
    atom_out_b = np.asarray(atom_out_b, dtype=np.float32)

    bf = ml_dtypes.bfloat16

    # ---- host routing (tiny) ----
    cls3 = x[:, :NCLS, :]                                   # [B, 6, D]
    logits = np.einsum("bnd,nd->bn", cls3, gate_delta)      # [B, 6] f32
    choose_left = logits >= 0
    p_left = _sigmoid(logits)
    wgt = np.where(choose_left, p_left, 1.0 - p_left).astype(np.float32)
    keys = np.where(choose_left, LEFT_KEYS[None, :], RIGHT_KEYS[None, :])
    src = (keys // NA).reshape(-1)                          # [384]
    dst = (keys % NA).reshape(-1)
    wflat = wgt.reshape(-1)                                 # [384]

    # permute cls tokens by (src, dst): src groups contiguous, dst groups
    # become a few contiguous ranges
    order = np.lexsort((dst, src))
    inv_order = np.argsort(order)
    src_p, dst_p, wflat_p = src[order], dst[order], wflat[order]
    goff = tuple(int(np.searchsorted(src_p, s)) for s in range(NA + 1))
    dranges = []
    for a in range(NA):
        idx = np.flatnonzero(dst_p == a)
        ranges = []
        if idx.size:
            brk = np.flatnonzero(np.diff(idx) > 1)
            starts = np.concatenate(([0], brk + 1))
            ends = np.concatenate((brk, [idx.size - 1]))
            ranges = [(int(idx[s]), int(idx[e]) + 1)
                      for s, e in zip(starts, ends)]
        dranges.append(tuple(ranges))
    dranges = tuple(dranges)

    wrep_rep = np.ascontiguousarray(
        np.broadcast_to(wflat_p.reshape(1, NT), (128, NT))).astype(bf)

    # ---- replicated tensors ----
    # clsT[p, d*NT + t] = cls_permuted[t, d*128+p]
    clsT = np.ascontiguousarray(
        cls3.reshape(NT, D)[order].reshape(NT, DT, 128).transpose(2, 1, 0)
    ).reshape(128, DT * NT).astype(bf)
    b1T = np.ascontiguousarray(patch_b1.reshape(HT, 128).T)

    # w1P[p, q, s, hh, g, i, m] = w1{hi/lo}[(q*HPP+hh)*128+m, g*256+i*128+p]
    w1h, w1l = _split8(patch_w1 * WSC)
    w1P = np.ascontiguousarray(
        np.stack([w1h.reshape(NW1P, HPP, 128, NG, 2, 128),
                  w1l.reshape(NW1P, HPP, 128, NG, 2, 128)], axis=0)
        .transpose(6, 1, 0, 2, 4, 5, 3))
    # w2P[p, hf, s, dp', j, i, m] = w2{hi/lo}[(hf*3+dp')*128+m, (2j+i)*128+p]
    w2h, w2l = _split8(patch_w2 * WSC)
    w2P = np.ascontiguousarray(
        np.stack([w2h.reshape(2, 3, 128, NJ, 2, 128),
                  w2l.reshape(2, 3, 128, NJ, 2, 128)], axis=0)
        .transpose(6, 1, 0, 2, 4, 5, 3))

    # ---- per-core tensors ----
    patch = x[:, NCLS:, :].reshape(NCORES, TPC, D)
    # xP[c][p, ci, s, g, i, t] = x{hi/lo}[tok ci*CW+t, g*256+i*128+p]
    xr = patch.reshape(NCORES, NCH, CW, NG, 2, 128)
    xh, xl = _split8(xr)
    xP_all = np.ascontiguousarray(
        np.stack([xh, xl], axis=3).transpose(0, 6, 1, 3, 4, 5, 2))

    ainT_all, ainbT_all, aoutT_all = [], [], []
    for c in range(NCORES):
        hsl = slice(HSH * c, HSH * (c + 1))
        ainT = np.ascontiguousarray(
            atom_in_w[:, hsl, :].reshape(NA, KPA, 128, DT, 128)
            .transpose(3, 4, 0, 1, 2)).reshape(DT, 128, NA * HSH).astype(bf)
        ainT_all.append(ainT)
        ainbT_all.append(np.ascontiguousarray(
            atom_in_b[:, hsl].reshape(HLT, 128).T))
        aoutT = np.ascontiguousarray(
            atom_out_w[:, :, hsl].reshape(NA, DT, 128, KPA, 128)
            .transpose(0, 4, 3, 1, 2)).reshape(NA, 128, KPA * D).astype(bf)
        aoutT_all.append(aoutT)

    in_maps = []
    for c in range(NCORES):
        in_maps.append({
            "xP": xP_all[c], "w1P": w1P, "b1T": b1T, "w2P": w2P,
            "clsT": clsT, "ainT": ainT_all[c], "ainbT": ainbT_all[c],
            "aoutT": aoutT_all[c], "wrep": wrep_rep,
        })

    key = (goff, dranges)
    nc = _CACHE.get(key)
    if nc is None:
        nc = _build_program(goff, dranges)
        _CACHE[key] = nc

    res = run_bass_kernel_spmd(nc, in_maps, core_ids=list(range(NCORES)))
    global LAST_RESULTS
    LAST_RESULTS = res

    # ---- host gather ----
    patch_out = np.empty((B, P, D), dtype=np.float32)
    for c in range(NCORES):
        poutT = res.results[c]["poutT"].reshape(D, TPC).astype(np.float32)
        patch_out[BPC * c:BPC * (c + 1)] = (
            poutT.T + patch_b2[None, :]).reshape(BPC, P, D)

    cpart = np.zeros((D, NT), dtype=np.float32)
    for c in range(NCORES):
        cpart += res.results[c]["cpartT"].reshape(D, NT).astype(np.float32)
    cls_out = cpart.T[inv_order] + wflat[:, None] * atom_out_b[dst, :]
    cls_out = cls_out.reshape(B, NCLS, D)

    return np.concatenate([cls_out, patch_out], axis=1)


# revision 36
# speedup vs baseline: 1.4104x; 1.4104x over previous
"""Trainium2 Bass kernel for nn_Mlp_moe: dense patch-token MLP + top-1 gated
atom (expert) routing for 6 CLS task tokens.

Sharding over 8 NeuronCores:
  - Patch MLP: data-parallel over batch B=64 -> 8 batches (1568 patch tokens)
    per core. MLP weights replicated (SBUF-resident, bf16).
  - Atom/CLS part: hidden dim H=3072 sharded 8-way (384 per core); every core
    processes all 384 CLS tokens for all 5 atoms on its H-shard and emits a
    partial output summed on the host. Routing (gate logits/sigmoid/top-1
    masks) is computed on the host (it is O(B*6*D), negligible) and shipped
    as {0,1}/weight masks folded into the device compute.

Schedule (v2): the PE is warmed with dummy matmuls from program start (HAM
clock gate releases after ~3.4us of activity), while DMAs stream the first
working set (w1 piece 0 + x chunk 0) on need-ordered queues. w1 arrives in 8
pieces so GEMM1 starts as soon as the first 3 h-tiles + x0 land. Atom phases
run between chunk0's GEMM1 and GEMM2. Outputs stage through SBUF as bf16.

Device compute is bf16 (PSUM accumulation is fp32; erf-Gelu on ScalarE is
~exact); patch outputs are bf16, cls partials fp32.
"""

import numpy as np
import ml_dtypes

import concourse.bass as bass
import concourse.bacc as bacc
import concourse.mybir as mybir
from concourse import tile
from concourse.bass_utils import run_bass_kernel_spmd

NCORES = 8
B, NCLS, P, D, H = 64, 6, 196, 768, 3072
NA = 5
HSH = H // NCORES            # 384: per-core atom hidden shard
BPC = B // NCORES            # 8 batches per core
TPC = BPC * P                # 1568 patch tokens per core
NT = B * NCLS                # 384 cls tokens
DT = D // 128                # 6 d-tiles
HT = H // 128                # 24 h-tiles
HLT = NA * HSH // 128        # 15 atom h-shard tiles (a-major, 3 per atom)
KPA = HSH // 128             # 3 h-shard tiles per atom
CW = 392
NCH = 4
CHUNKS = [(i * CW, CW) for i in range(NCH)]
NW1P = 8                     # w1 DMA pieces (3 h-tiles each)
HPP = HT // NW1P             # 3 h-tiles per piece
NWARM = 52                   # PE warm-up dummy matmuls

LEFT_KEYS = np.array([3, 4, 8, 9, 13, 14], dtype=np.int64)
RIGHT_KEYS = np.array([15, 20, 16, 21, 17, 22], dtype=np.int64)

BF16 = mybir.dt.bfloat16
F32 = mybir.dt.float32
AF = mybir.ActivationFunctionType

_CACHE = {}
LAST_RESULTS = None  # BassKernelResults of the most recent run (for profiling)


def _build_program(goff, dranges):
    """goff: 6 cumulative offsets of the 5 src-atom token groups (cls tokens
    are host-permuted by (src, dst) so each atom's tokens are a contiguous
    column range). dranges[a]: list of (start, end) column ranges whose
    tokens route their output through atom a.
    """
    nc = bacc.Bacc(None, target_bir_lowering=False, debug=False,
                   num_devices=NCORES)

    # partition-major packed inputs (see host layouts in kernel())
    xT_d = nc.dram_tensor("xT", [128, NCH * DT * CW], BF16,
                          kind="ExternalInput")
    w1T_d = nc.dram_tensor("w1T", [128, NW1P, DT * HPP * 128], BF16,
                           kind="ExternalInput")
    b1T_d = nc.dram_tensor("b1T", [128, HT], F32, kind="ExternalInput")
    w2T_d = nc.dram_tensor("w2T", [128, HT * D], BF16, kind="ExternalInput")
    clsT_d = nc.dram_tensor("clsT", [128, DT * NT], BF16,
                            kind="ExternalInput")
    ainT_d = nc.dram_tensor("ainT", [DT, 128, NA * HSH], BF16,
                            kind="ExternalInput")
    ainbT_d = nc.dram_tensor("ainbT", [128, HLT], F32, kind="ExternalInput")
    aoutT_d = nc.dram_tensor("aoutT", [NA, 128, KPA * D], BF16,
                             kind="ExternalInput")
    wrep_d = nc.dram_tensor("wrep", [128, NT], BF16, kind="ExternalInput")
    poutT_d = nc.dram_tensor("poutT", [DT, 128, TPC], BF16,
                             kind="ExternalOutput")
    cpartT_d = nc.dram_tensor("cpartT", [DT, 128, NT], BF16,
                              kind="ExternalOutput")

    with tile.TileContext(nc) as tc:
        with (
            tc.tile_pool(name="w", bufs=1) as wp,
            tc.tile_pool(name="gat", bufs=1) as gp,
            tc.tile_pool(name="hida", bufs=1) as hp,
            tc.tile_pool(name="xin", bufs=2) as xp,
            tc.tile_pool(name="g1", bufs=48) as g1p,
            tc.tile_pool(name="ostg", bufs=4) as op,
            tc.tile_pool(name="ps", bufs=7, space="PSUM") as pp,
            tc.tile_pool(name="psw", bufs=1, space="PSUM") as pwp,
        ):
            # ---- PE warm-up: dummy matmuls from program start ----
            # The HAM clock gate holds the PE at 1.2 GHz until ~3.4us of
            # sustained activity; these dummies run while the first DMAs
            # stream in so the real matmuls start at 2.4 GHz.
            wdum = wp.tile([128, 512], BF16, tag="wdum", name="wdum")
            nc.vector.memset(wdum[:], 0.03125)
            pdum = pwp.tile([128, 256], F32, tag="pdum", name="pdum")
            for _ in range(NWARM):
                nc.tensor.matmul(pdum[:], wdum[:, :128], wdum[:, :256],
                                 start=True, stop=True)

            # ---- DMA issues: global need order striped over the 3 DMA
            # queues (sync/scalar/gpsimd) so the heads of all queues are
            # always the next-needed tensors and the shared DGE engine pool
            # serves the critical path first.
            def load_x(ci):
                xa = xp.tile([128, DT * CW], BF16, tag="x", name="x")
                nc.sync.dma_start(
                    xa[:], xT_d[:, ci * DT * CW:(ci + 1) * DT * CW])
                return xa

            w1T = [wp.tile([128, DT * HPP * 128], BF16, tag=f"w1{q}",
                           name=f"w1{q}") for q in range(NW1P)]
            b1T = wp.tile([128, HT], F32, tag="b1", name="b1")
            clsT = wp.tile([128, DT * NT], BF16, tag="cls", name="cls")
            w2T = wp.tile([128, HT * D], BF16, tag="w2", name="w2")
            ainbT = wp.tile([128, HLT], F32, tag="ainb", name="ainb")
            ainT = [wp.tile([128, NA * HSH], BF16, tag=f"ain{d}",
                            name=f"ain{d}") for d in range(DT)]
            wrep = wp.tile([128, NT], BF16, tag="wr", name="wr")
            aoutT = [wp.tile([128, KPA * D], BF16, tag=f"ao{a}",
                             name=f"ao{a}") for a in range(NA)]

            # wave 1: chunk0 GEMM1 working set. x0 and every w1 piece are
            # split in d-thirds round-robined over all three queues, so the
            # shared DGE pool delivers them in exact need order at full
            # aggregate bandwidth (piece k lands before the matmuls for
            # piece k-1 complete).
            # NOTE: an engine's next dma_start blocks until its previous
            # transfer completes, so ScalarE (which must run the gelus from
            # ~16us on) gets only the 3 earliest DMAs; sync/gpsimd (no
            # compute duties) carry everything else.
            xa0 = xp.tile([128, DT * CW], BF16, tag="x", name="x")
            qs = [nc.sync, nc.scalar, nc.gpsimd]
            for i, q in enumerate(qs):
                q.dma_start(xa0[:, i * 2 * CW:(i + 1) * 2 * CW],
                            xT_d[:, i * 2 * CW:(i + 1) * 2 * CW])
            xs_pre = [xa0]
            nc.scalar.dma_start(b1T[:], b1T_d[:])
            w1c = DT * HPP * 128
            nc.scalar.dma_start(w1T[0][:, :w1c // 3], w1T_d[:, 0, :w1c // 3])
            nc.sync.dma_start(w1T[0][:, w1c // 3:2 * w1c // 3],
                              w1T_d[:, 0, w1c // 3:2 * w1c // 3])
            nc.gpsimd.dma_start(w1T[0][:, 2 * w1c // 3:],
                                w1T_d[:, 0, 2 * w1c // 3:])
            for p in range(1, NW1P):
                h1, h2 = (nc.sync, nc.gpsimd) if p % 2 else \
                    (nc.gpsimd, nc.sync)
                h1.dma_start(w1T[p][:, :w1c // 2], w1T_d[:, p, :w1c // 2])
                h2.dma_start(w1T[p][:, w1c // 2:], w1T_d[:, p, w1c // 2:])
            # wave 2: chunk1 x + atom-in tensors
            xs_pre.append(load_x(1))                           # sync
            nc.gpsimd.dma_start(ainT[0][:], ainT_d[0])
            nc.sync.dma_start(clsT[:], clsT_d[:])
            nc.gpsimd.dma_start(ainT[1][:], ainT_d[1])
            nc.sync.dma_start(ainT[2][:], ainT_d[2])
            nc.gpsimd.dma_start(ainT[3][:], ainT_d[3])
            nc.sync.dma_start(ainT[4][:], ainT_d[4])
            nc.gpsimd.dma_start(ainT[5][:], ainT_d[5])
            nc.sync.dma_start(ainbT[:], ainbT_d[:])
            # wave 3: w2 + gate weights + atom-out tensors
            nc.gpsimd.dma_start(w2T[:, :12 * D], w2T_d[:, :12 * D])
            nc.sync.dma_start(w2T[:, 12 * D:], w2T_d[:, 12 * D:])
            nc.gpsimd.dma_start(wrep[:], wrep_d[:])
            nc.sync.dma_start(aoutT[0][:], aoutT_d[0])
            nc.gpsimd.dma_start(aoutT[1][:], aoutT_d[1])
            nc.sync.dma_start(aoutT[2][:], aoutT_d[2])
            nc.gpsimd.dma_start(aoutT[3][:], aoutT_d[3])
            nc.sync.dma_start(aoutT[4][:], aoutT_d[4])

            # ---- patch GEMM1 for one chunk (piece-gated on first chunk) ----
            def patch_g1(ci, xa):
                c0, cw = CHUNKS[ci]
                g1s = []
                for h in range(HT):
                    ps = pp.tile([128, 512], F32, tag="ps", name="ps")
                    q, hh = divmod(h, HPP)
                    for d in range(DT):
                        nc.tensor.matmul(
                            ps[:, :cw],
                            w1T[q][:, d * HPP * 128 + hh * 128:
                                   d * HPP * 128 + (hh + 1) * 128],
                            xa[:, d * CW:d * CW + cw],
                            start=(d == 0), stop=(d == DT - 1))
                    g1 = g1p.tile([128, CW], BF16, tag="g1", name="g1")
                    nc.scalar.activation(g1[:, :cw], ps[:, :cw], AF.Gelu,
                                         bias=b1T[:, h:h + 1])
                    g1s.append(g1)
                return g1s

            def patch_g2(ci, g1s):
                c0, cw = CHUNKS[ci]
                for dp in range(DT):
                    ps = pp.tile([128, 512], F32, tag="ps", name="ps")
                    for h in range(HT):
                        nc.tensor.matmul(
                            ps[:, :cw],
                            w2T[:, h * D + dp * 128:h * D + (dp + 1) * 128],
                            g1s[h][:, :cw],
                            start=(h == 0), stop=(h == HT - 1))
                    stg = op.tile([128, CW], BF16, tag="ostg", name="ostg")
                    nc.vector.tensor_copy(stg[:, :cw], ps[:, :cw])
                    nc.gpsimd.dma_start(poutT_d[dp][:, c0:c0 + cw],
                                        stg[:, :cw])

            g1s_c0 = patch_g1(0, xs_pre[0])
            g1s_c1 = patch_g1(1, xs_pre[1])

            # ---- phase A: grouped atom in-GEMM + gelu ----
            # cls tokens are host-permuted by src atom: group s occupies
            # columns [goff[s], goff[s+1]), so each token's hidden state is
            # computed only for its routed atom (1/5 the FLOPs of the dense
            # all-atom form).
            Gk = [gp.tile([128, NT], BF16, tag=f"g{k}", name=f"g{k}")
                  for k in range(KPA)]
            for s in range(NA):
                o0, o1 = goff[s], goff[s + 1]
                ns = o1 - o0
                if ns == 0:
                    continue
                for k in range(KPA):
                    ps = pp.tile([128, 512], F32, tag="ps", name="ps")
                    c0 = s * HSH + k * 128
                    for d in range(DT):
                        nc.tensor.matmul(
                            ps[:, :ns],
                            ainT[d][:, c0:c0 + 128],
                            clsT[:, d * NT + o0:d * NT + o1],
                            start=(d == 0), stop=(d == DT - 1))
                    nc.scalar.activation(Gk[k][:, o0:o1], ps[:, :ns],
                                         AF.Gelu,
                                         bias=ainbT[:, s * KPA + k:
                                                    s * KPA + k + 1])

            # ---- phase B: scale hidden by the gate weight (DVE) ----
            Hk = []
            for k in range(KPA):
                h = hp.tile([128, NT], BF16, tag=f"hid{k}", name=f"hid{k}")
                nc.vector.tensor_mul(h[:], Gk[k][:], wrep[:])
                Hk.append(h)

            patch_g2(0, g1s_c0)

            # ---- atom out-GEMM, grouped by dst atom ----
            # Each column range in dranges[a] holds tokens routed to atom a;
            # each range accumulates over the KPA h-shard tiles only.
            # PSUM zero regions are whole banks: the FIRST matmul into the
            # tile carries start=True (lazily zeroing the bank); every other
            # matmul accumulates — first touch of a pending byte zeroes it.
            nmm_out = sum(KPA * len(dranges[a]) for a in range(NA))
            for dp in range(DT):
                ps = pp.tile([128, 512], F32, tag="ps", name="ps")
                n = 0
                for a in range(NA):
                    for k in range(KPA):
                        for (r0, r1) in dranges[a]:
                            nc.tensor.matmul(
                                ps[:, r0:r1],
                                aoutT[a][:, k * D + dp * 128:
                                         k * D + (dp + 1) * 128],
                                Hk[k][:, r0:r1],
                                start=(n == 0), stop=(n == nmm_out - 1),
                                skip_group_check=True)
                            n += 1
                stg = op.tile([128, CW], BF16, tag="cstg", name="cstg")
                nc.vector.tensor_copy(stg[:, :NT], ps[:, :NT])
                nc.gpsimd.dma_start(cpartT_d[dp], stg[:, :NT])

            # ---- patch chunks 1..3 ----
            patch_g2(1, g1s_c1)
            xs_pre.append(load_x(2))
            patch_g2(2, patch_g1(2, xs_pre[2]))
            xs_pre.append(load_x(3))
            patch_g2(3, patch_g1(3, xs_pre[3]))

    nc.compile()
    return nc


def _sigmoid(x):
    out = np.empty_like(x)
    pos = x >= 0
    out[pos] = 1.0 / (1.0 + np.exp(-x[pos]))
    ex = np.exp(x[~pos])
    out[~pos] = ex / (1.0 + ex)
    return out


def kernel(x, patch_w1, patch_b1, patch_w2, patch_b2, gate_delta,
           atom_in_w, atom_in_b, atom_out_w, atom_out_b):
    x = np.asarray(x, dtype=np.float32)
    patch_w1 = np.asarray(patch_w1, dtype=np.float32)
    patch_b1 = np.asarray(patch_b1, dtype=np.float32)
    patch_w2 = np.asarray(patch_w2, dtype=np.float32)
    patch_b2 = np.asarray(patch_b2, dtype=np.float32)
    gate_delta = np.asarray(gate_delta, dtype=np.float32)
    atom_in_w = np.asarray(atom_in_w, dtype=np.float32)
    atom_in_b = np.asarray(atom_in_b, dtype=np.float32)
    atom_out_w = np.asarray(atom_out_w, dtype=np.float32)
    atom_out_b = np.asarray(atom_out_b, dtype=np.float32)

    bf = ml_dtypes.bfloat16

    # ---- host routing (tiny) ----
    cls3 = x[:, :NCLS, :]                                   # [B, 6, D]
    logits = np.einsum("bnd,nd->bn", cls3, gate_delta)      # [B, 6] f32
    choose_left = logits >= 0
    p_left = _sigmoid(logits)
    wgt = np.where(choose_left, p_left, 1.0 - p_left).astype(np.float32)
    keys = np.where(choose_left, LEFT_KEYS[None, :], RIGHT_KEYS[None, :])
    src = (keys // NA).reshape(-1)                          # [384]
    dst = (keys % NA).reshape(-1)
    wflat = wgt.reshape(-1)                                 # [384]

    # permute cls tokens by (src, dst) so each src atom's tokens are
    # contiguous and each dst atom's tokens are a few contiguous ranges
    order = np.lexsort((dst, src))
    inv_order = np.argsort(order)
    src_p, dst_p, wflat_p = src[order], dst[order], wflat[order]
    goff = tuple(int(np.searchsorted(src_p, s)) for s in range(NA + 1))
    dranges = []
    for a in range(NA):
        idx = np.flatnonzero(dst_p == a)
        ranges = []
        if idx.size:
            brk = np.flatnonzero(np.diff(idx) > 1)
            starts = np.concatenate(([0], brk + 1))
            ends = np.concatenate((brk, [idx.size - 1]))
            ranges = [(int(idx[s]), int(idx[e]) + 1)
                      for s, e in zip(starts, ends)]
        dranges.append(tuple(ranges))
    dranges = tuple(dranges)

    wrep_rep = np.ascontiguousarray(
        np.broadcast_to(wflat_p.reshape(1, NT), (128, NT))).astype(bf)

    # ---- replicated tensors (partition-major packed) ----
    # clsT[p, d*NT + t] = cls_permuted[t, d*128+p]
    clsT = np.ascontiguousarray(
        cls3.reshape(NT, D)[order].reshape(NT, DT, 128).transpose(2, 1, 0)
    ).reshape(128, DT * NT).astype(bf)
    # w1T[p, q, d*384 + hh*128 + m] = patch_w1[(q*3+hh)*128+m, d*128+p]
    w1T = np.ascontiguousarray(
        patch_w1.reshape(NW1P, HPP, 128, DT, 128).transpose(4, 0, 3, 1, 2)
    ).reshape(128, NW1P, DT * HPP * 128).astype(bf)
    b1T = np.ascontiguousarray(patch_b1.reshape(HT, 128).T)
    # w2T[p, h*D + dp*128 + m] = patch_w2[dp*128+m, h*128+p]
    w2T = np.ascontiguousarray(
        patch_w2.reshape(DT, 128, HT, 128).transpose(3, 2, 0, 1)
    ).reshape(128, HT * D).astype(bf)

    # ---- per-core tensors ----
    patch = x[:, NCLS:, :].reshape(NCORES, TPC, D)
    # xT[p, ci*DT*CW + d*CW + t] = patch[c][ci*CW+t, d*128+p]
    xT_all = np.ascontiguousarray(
        patch.reshape(NCORES, NCH, CW, DT, 128).transpose(0, 4, 1, 3, 2)
    ).reshape(NCORES, 128, NCH * DT * CW).astype(bf)

    ainT_all, ainbT_all, aoutT_all = [], [], []
    for c in range(NCORES):
        hsl = slice(HSH * c, HSH * (c + 1))
        # ainT[d, p, a*HSH + k*128 + m] = atom_in_w[a, hsl0 + k*128+m, d*128+p]
        ainT = np.ascontiguousarray(
            atom_in_w[:, hsl, :].reshape(NA, KPA, 128, DT, 128)
            .transpose(3, 4, 0, 1, 2)).reshape(DT, 128, NA * HSH).astype(bf)
        ainT_all.append(ainT)
        ainbT_all.append(np.ascontiguousarray(
            atom_in_b[:, hsl].reshape(HLT, 128).T))
        # aoutT[a, p, k*D + dp*128 + m] = atom_out_w[a, dp*128+m, hsl0+k*128+p]
        aoutT = np.ascontiguousarray(
            atom_out_w[:, :, hsl].reshape(NA, DT, 128, KPA, 128)
            .transpose(0, 4, 3, 1, 2)).reshape(NA, 128, KPA * D).astype(bf)
        aoutT_all.append(aoutT)

    in_maps = []
    for c in range(NCORES):
        in_maps.append({
            "xT": xT_all[c], "w1T": w1T, "b1T": b1T, "w2T": w2T,
            "clsT": clsT, "ainT": ainT_all[c], "ainbT": ainbT_all[c],
            "aoutT": aoutT_all[c], "wrep": wrep_rep,
        })

    key = (goff, dranges)
    nc = _CACHE.get(key)
    if nc is None:
        nc = _build_program(goff, dranges)
        _CACHE[key] = nc

    res = run_bass_kernel_spmd(nc, in_maps, core_ids=list(range(NCORES)))
    global LAST_RESULTS
    LAST_RESULTS = res

    # ---- host gather ----
    patch_out = np.empty((B, P, D), dtype=np.float32)
    for c in range(NCORES):
        poutT = res.results[c]["poutT"].reshape(D, TPC).astype(np.float32)
        patch_out[BPC * c:BPC * (c + 1)] = (
            poutT.T + patch_b2[None, :]).reshape(BPC, P, D)

    cpart = np.zeros((D, NT), dtype=np.float32)
    for c in range(NCORES):
        cpart += res.results[c]["cpartT"].reshape(D, NT).astype(np.float32)
    cls_out = cpart.T[inv_order] + wflat[:, None] * atom_out_b[dst, :]
    cls_out = cls_out.reshape(B, NCLS, D)

    return np.concatenate([cls_out, patch_out], axis=1)
